# revision 9
# baseline (speedup 1.0000x reference)
"""Causal self-attention (GQA, RoPE, QK-RMSNorm) Trainium2 Bass kernel.

Sharding: tensor-parallel over heads x data-parallel over batch.
8 cores = 2 batch-groups (2 batches each) x 4 head-groups
(4 q heads + 1 kv head per core, GQA nrep=4).

Per core:
  - phase A: QKV projection (x^T via DMA-transpose as stationary operand),
    RoPE + RMSNorm in natural [t, d] layout, PE-transpose q,k to [d, t].
  - phase B: per (batch, head): s^T = k^T.T @ q^T tiles -> exp on ACT
    (no max subtraction; rmsnormed q,k bound |scores| <= sqrt(128)) ->
    causal mask by multiply on diagonal tiles -> y = e^T.T @ [v|1]
    (ones column gives softmax denominators for free) -> normalize by
    reciprocal -> PE-transpose to y^T.
  - phase C: partial out = y^T.T @ Wo_shard -> DRAM.
Host sums the 4 head-group partials per batch-group.

All matmuls bf16 inputs / fp32 PSUM accumulation; softmax, RoPE and
RMSNorm arithmetic in fp32. rsqrt computed as exp(-0.5*ln(x)) so every
ACT call stays in the natural_log_exp table set (no table thrashing).
"""

import sys
from contextlib import ExitStack

import numpy as np

sys.path.insert(0, "/opt/trn_rl_repo")

import concourse.bass as bass  # noqa: E402
import concourse.mybir as mybir  # noqa: E402
import concourse.tile as tile  # noqa: E402
from concourse import bacc  # noqa: E402

FP32 = mybir.dt.float32
BF16 = mybir.dt.bfloat16
P = 128
HD = 128
HD2 = HD // 2
VW = 160  # v_sb row width: 128 v cols + ones col + pad
TQW = 512  # tq chunk width for the QK stage


def build_nc(T=2048, C=2048, NHL=4, BL=2):
    """Build the per-core Bass program. Same program on all 8 cores."""
    TL = BL * T
    NCC = C // P  # contraction chunks
    NTB = T // P  # t-128 chunks per batch
    DQ = NHL * HD
    NJ = T // TQW  # tq-512 chunks per batch
    NK = T // P  # tk-128 chunks per batch
    NCO = C // 512
    sm_scale = float(1.0 / np.sqrt(HD))
    EPS = 1e-6

    nc = bacc.Bacc(None)
    x = nc.declare_dram_parameter("x", [C, TL], BF16, isOutput=False)
    wq = nc.declare_dram_parameter("wq", [C, DQ], BF16, isOutput=False)
    wk = nc.declare_dram_parameter("wk", [C, HD], BF16, isOutput=False)
    wv = nc.declare_dram_parameter("wv", [C, HD], BF16, isOutput=False)
    wo = nc.declare_dram_parameter("wo", [DQ, C], BF16, isOutput=False)
    cs = nc.declare_dram_parameter("cs", [T, HD], FP32, isOutput=False)
    sc = nc.declare_dram_parameter("sc", [T, HD], FP32, isOutput=False)
    masks = nc.declare_dram_parameter("masks", [4, P, TQW], BF16, isOutput=False)
    ident = nc.declare_dram_parameter("ident", [P, P], BF16, isOutput=False)
    out = nc.declare_dram_parameter("out", [TL, C], FP32, isOutput=True)

    Exp = mybir.ActivationFunctionType.Exp
    Ln = mybir.ActivationFunctionType.Ln
    xr = x.rearrange("(n p) t -> p n t", p=P)  # x is shipped pre-transposed [C, TL]

    with tile.TileContext(nc) as tc, ExitStack() as ctx:
        const = ctx.enter_context(tc.tile_pool(name="const", bufs=1))
        persist = ctx.enter_context(tc.tile_pool(name="persist", bufs=1))
        xt_pool = ctx.enter_context(tc.tile_pool(name="xt", bufs=3))
        work = ctx.enter_context(tc.tile_pool(name="work", bufs=2))
        et_pool = ctx.enter_context(tc.tile_pool(name="et", bufs=4 * NJ + 2))
        outsb = ctx.enter_context(tc.tile_pool(name="outsb", bufs=3))
        ps_mm = ctx.enter_context(tc.tile_pool(name="psmm", bufs=4, space="PSUM"))
        ps_s = ctx.enter_context(tc.tile_pool(name="pss", bufs=2, space="PSUM"))
        ps_y = ctx.enter_context(tc.tile_pool(name="psy", bufs=2, space="PSUM"))

        # ---- constants / persistent state ----
        wq_sb = const.tile([P, NCC, DQ], BF16)
        nc.sync.dma_start(wq_sb[:], wq.rearrange("(n p) d -> p n d", p=P))
        wk_sb = const.tile([P, NCC, HD], BF16)
        nc.sync.dma_start(wk_sb[:], wk.rearrange("(n p) d -> p n d", p=P))
        wv_sb = const.tile([P, NCC, HD], BF16)
        nc.sync.dma_start(wv_sb[:], wv.rearrange("(n p) d -> p n d", p=P))
        wo_sb = const.tile([P, NHL, C], BF16)
        nc.sync.dma_start(wo_sb[:], wo.rearrange("(h p) c -> p h c", p=P))
        cs_sb = const.tile([P, NTB, HD], FP32)
        nc.sync.dma_start(cs_sb[:], cs.rearrange("(n p) d -> p n d", p=P))
        sc_sb = const.tile([P, NTB, HD], FP32)
        nc.sync.dma_start(sc_sb[:], sc.rearrange("(n p) d -> p n d", p=P))
        mask_sb = const.tile([P, 4, TQW], BF16)
        nc.sync.dma_start(mask_sb[:], masks.rearrange("m p f -> p m f"))
        ident_sb = const.tile([P, P], BF16)
        nc.sync.dma_start(ident_sb[:], ident[:])
        eps_sb = const.tile([P, 1], FP32)
        nc.vector.memset(eps_sb[:], EPS)

        qT_sb = persist.tile([P, BL * NHL, T], BF16)  # [d, plane, t]
        kT_sb = persist.tile([P, BL, T], BF16)
        v_sb = persist.tile([P, BL, NK, VW], BF16)  # [tk_in, b, tk_chunk, d|1]
        yT_sb = persist.tile([P, BL * NHL, T], BF16)
        nc.vector.memset(v_sb[:, :, :, HD : HD + 4], 1.0)

        # ---- phase A: projections + rope + rmsnorm + transpose ----
        for b in range(BL):
            for t16 in range(NTB):
                trow = b * T + t16 * P
                xt = xt_pool.tile([P, NCC, P], BF16, tag="xt")
                nc.sync.dma_start(xt[:], xr[:, :, trow : trow + P])

                psq = ps_mm.tile([P, 512], FP32, tag="mm512")
                pskv = ps_mm.tile([P, 512], FP32, tag="mm512")
                for c in range(NCC):
                    nc.tensor.matmul(
                        psq[:, :DQ], xt[:, c, :], wq_sb[:, c, :],
                        start=(c == 0), stop=(c == NCC - 1),
                    )
                for c in range(NCC):
                    nc.tensor.matmul(
                        pskv[:, 0:HD], xt[:, c, :], wk_sb[:, c, :],
                        start=(c == 0), stop=(c == NCC - 1),
                    )
                for c in range(NCC):
                    nc.tensor.matmul(
                        pskv[:, HD : 2 * HD], xt[:, c, :], wv_sb[:, c, :],
                        start=(c == 0), stop=(c == NCC - 1),
                    )

                # rope: p1 = [x1*cos | x2*sin], p2 = [x1*sin | x2*cos]
                # y1 = p1_lo + p1_hi ; y2 = p2_hi - p2_lo
                psq3 = psq[:, 0:DQ].rearrange("p (h d) -> p h d", d=HD)
                csb = cs_sb[:, t16, None, :].to_broadcast((P, NHL, HD))
                scb = sc_sb[:, t16, None, :].to_broadcast((P, NHL, HD))
                p1 = work.tile([P, NHL, HD], FP32, tag="p1")
                p2 = work.tile([P, NHL, HD], FP32, tag="p2")
                nc.vector.tensor_mul(p1[:], psq3, csb)
                nc.vector.tensor_mul(p2[:], psq3, scb)
                q_ro = work.tile([P, NHL, HD], FP32, tag="qro")
                nc.vector.tensor_add(
                    q_ro[:, :, 0:HD2], p1[:, :, 0:HD2], p1[:, :, HD2:HD]
                )
                nc.vector.tensor_sub(
                    q_ro[:, :, HD2:HD], p2[:, :, HD2:HD], p2[:, :, 0:HD2]
                )
                p1k = work.tile([P, HD], FP32, tag="p1k")
                p2k = work.tile([P, HD], FP32, tag="p2k")
                nc.vector.tensor_mul(p1k[:], pskv[:, 0:HD], cs_sb[:, t16, :])
                nc.vector.tensor_mul(p2k[:], pskv[:, 0:HD], sc_sb[:, t16, :])
                k_ro = work.tile([P, HD], FP32, tag="kro")
                nc.vector.tensor_add(k_ro[:, 0:HD2], p1k[:, 0:HD2], p1k[:, HD2:HD])
                nc.vector.tensor_sub(k_ro[:, HD2:HD], p2k[:, HD2:HD], p2k[:, 0:HD2])

                # rmsnorm scales: rs = exp(-0.5 * ln(ssq/HD + eps))
                sq = work.tile([P, NHL, HD], FP32, tag="sq")
                nc.vector.tensor_mul(sq[:], q_ro[:], q_ro[:])
                sqk = work.tile([P, HD], FP32, tag="sqk")
                nc.vector.tensor_mul(sqk[:], k_ro[:], k_ro[:])
                ssq = work.tile([P, NHL + 1], FP32, tag="ssq")
                nc.vector.tensor_reduce(
                    ssq[:, 0:NHL], sq[:], axis=mybir.AxisListType.X,
                    op=mybir.AluOpType.add,
                )
                nc.vector.tensor_reduce(
                    ssq[:, NHL : NHL + 1], sqk[:], axis=mybir.AxisListType.X,
                    op=mybir.AluOpType.add,
                )
                lnv = work.tile([P, NHL + 1], FP32, tag="lnv")
                nc.scalar.activation(
                    lnv[:], ssq[:], Ln, bias=eps_sb[:, 0:1], scale=1.0 / HD
                )
                rs = work.tile([P, NHL + 1], FP32, tag="rs")
                nc.scalar.activation(rs[:], lnv[:], Exp, scale=-0.5)

                q_n = work.tile([P, NHL, HD], BF16, tag="qn")
                for h in range(NHL):
                    nc.vector.tensor_scalar_mul(
                        q_n[:, h, :], q_ro[:, h, :], rs[:, h : h + 1]
                    )
                k_n = work.tile([P, HD], BF16, tag="kn")
                nc.vector.tensor_scalar_mul(k_n[:], k_ro[:], rs[:, NHL : NHL + 1])

                nc.vector.tensor_copy(v_sb[:, b, t16, 0:HD], pskv[:, HD : 2 * HD])

                for h in range(NHL):
                    pt = ps_y.tile([P, 132], BF16, tag="yt")
                    nc.tensor.transpose(pt[:, 0:P], q_n[:, h, :], ident_sb[:])
                    nc.any.tensor_copy(
                        qT_sb[:, b * NHL + h, t16 * P : (t16 + 1) * P], pt[:, 0:P]
                    )
                pt = ps_y.tile([P, 132], BF16, tag="yt")
                nc.tensor.transpose(pt[:, 0:P], k_n[:], ident_sb[:])
                nc.any.tensor_copy(kT_sb[:, b, t16 * P : (t16 + 1) * P], pt[:, 0:P])

        # ---- phase B: attention per (batch, head) ----
        for b in range(BL):
            for h in range(NHL):
                plane = b * NHL + h
                for j in range(NJ):
                    nkj = 4 * j + 4  # tk chunks up to (incl.) diagonal block row
                    et_tiles = []
                    for t0 in range(nkj):
                        ps = ps_s.tile([P, TQW], FP32, tag="s")
                        nc.tensor.matmul(
                            ps[:],
                            kT_sb[:, b, t0 * P : (t0 + 1) * P],
                            qT_sb[:, plane, j * TQW : (j + 1) * TQW],
                            start=True, stop=True,
                        )
                        et = et_pool.tile([P, TQW], BF16, tag="et")
                        nc.scalar.activation(et[:], ps[:], Exp, scale=sm_scale)
                        m = t0 - 4 * j
                        if m >= 0:
                            nc.vector.tensor_mul(et[:], et[:], mask_sb[:, m, :])
                        et_tiles.append(et)
                    for i in range(4):
                        tqc = 4 * j + i  # global tq-128 chunk in this batch
                        psy = ps_y.tile([P, 132], FP32, tag="yt")
                        for t0 in range(tqc + 1):
                            nc.tensor.matmul(
                                psy[:, 0 : HD + 1],
                                et_tiles[t0][:, i * P : (i + 1) * P],
                                v_sb[:, b, t0, 0 : HD + 1],
                                start=(t0 == 0), stop=(t0 == tqc),
                            )
                        rcp = work.tile([P, 1], FP32, tag="rcp")
                        nc.vector.reciprocal(rcp[:], psy[:, HD : HD + 1])
                        y_n = work.tile([P, HD], BF16, tag="ynrm")
                        nc.vector.tensor_scalar_mul(y_n[:], psy[:, 0:HD], rcp[:])
                        pt = ps_y.tile([P, 132], BF16, tag="yt")
                        nc.tensor.transpose(pt[:, 0:P], y_n[:], ident_sb[:])
                        nc.any.tensor_copy(
                            yT_sb[:, plane, tqc * P : (tqc + 1) * P], pt[:, 0:P]
                        )

        # ---- phase C: out_partial = y^T.T @ Wo_shard ----
        for b in range(BL):
            for t16 in range(NTB):
                for co in range(NCO):
                    pso = ps_mm.tile([P, 512], FP32, tag="mm512")
                    for h in range(NHL):
                        nc.tensor.matmul(
                            pso[:],
                            yT_sb[:, b * NHL + h, t16 * P : (t16 + 1) * P],
                            wo_sb[:, h, co * 512 : (co + 1) * 512],
                            start=(h == 0), stop=(h == NHL - 1),
                        )
                    o_sb = outsb.tile([P, 512], FP32, tag="osb")
                    nc.any.tensor_copy(o_sb[:], pso[:])
                    trow = b * T + t16 * P
                    nc.sync.dma_start(
                        out[trow : trow + P, co * 512 : (co + 1) * 512], o_sb[:]
                    )

    nc.finalize()
    return nc


def make_host_consts(T, dtype_bf):
    """Causal masks for the 4 diagonal-block offsets + PE-transpose identity."""
    i = np.arange(P)[:, None]
    j = np.arange(TQW)[None, :]
    masks = np.stack(
        [(i + P * m <= j).astype(np.float32) for m in range(4)]
    ).astype(dtype_bf)
    ident = np.eye(P, dtype=np.float32).astype(dtype_bf)
    return masks, ident


def prepare_in_maps(x, cos, sin, Wq, Wk, Wv, Wo, T=2048, C=2048, NHL=4, BL=2):
    import ml_dtypes

    bf = ml_dtypes.bfloat16
    B = x.shape[0]
    n_bgrp = B // BL
    n_hgrp = (Wq.shape[1] // HD) // NHL
    DQ = NHL * HD

    x_bf = np.ascontiguousarray(x.astype(bf))
    cosf = np.ascontiguousarray(cos.reshape(T, HD2).astype(np.float32))
    sinf = np.ascontiguousarray(sin.reshape(T, HD2).astype(np.float32))
    cs = np.ascontiguousarray(np.concatenate([cosf, sinf], axis=1))
    sc = np.ascontiguousarray(np.concatenate([sinf, cosf], axis=1))
    masks, ident = make_host_consts(T, bf)

    in_maps = []
    for g in range(n_bgrp):
        x_sh = np.ascontiguousarray(x_bf[BL * g : BL * (g + 1)].reshape(BL * T, C).T)
        for hg in range(n_hgrp):
            in_maps.append(
                {
                    "x": x_sh,
                    "wq": np.ascontiguousarray(
                        Wq[:, DQ * hg : DQ * (hg + 1)].astype(bf)
                    ),
                    "wk": np.ascontiguousarray(
                        Wk[:, HD * hg : HD * (hg + 1)].astype(bf)
                    ),
                    "wv": np.ascontiguousarray(
                        Wv[:, HD * hg : HD * (hg + 1)].astype(bf)
                    ),
                    "wo": np.ascontiguousarray(
                        Wo[DQ * hg : DQ * (hg + 1), :].astype(bf)
                    ),
                    "cs": cs,
                    "sc": sc,
                    "masks": masks,
                    "ident": ident,
                }
            )
    return in_maps


_RESULTS_CACHE = {}


def run_on_device(x, cos, sin, Wq, Wk, Wv, Wo, trace=False):
    from concourse.bass_utils import run_bass_kernel_spmd

    T, C, NHL, BL = 2048, 2048, 4, 2
    in_maps = prepare_in_maps(x, cos, sin, Wq, Wk, Wv, Wo, T, C, NHL, BL)
    nc = build_nc(T, C, NHL, BL)
    res = run_bass_kernel_spmd(nc, in_maps, list(range(8)), trace=trace)

    B = x.shape[0]
    out = np.zeros((B, T, C), np.float32)
    n_hgrp = len(in_maps) // (B // BL)
    for g in range(B // BL):
        acc = np.zeros((BL * T, C), np.float32)
        for hg in range(n_hgrp):
            acc += res.results[g * n_hgrp + hg]["out"]
        out[BL * g : BL * (g + 1)] = acc.reshape(BL, T, C)
    return out, res


def kernel(x, cos, sin, Wq, Wk, Wv, Wo):
    out, _ = run_on_device(
        np.asarray(x), np.asarray(cos), np.asarray(sin),
        np.asarray(Wq), np.asarray(Wk), np.asarray(Wv), np.asarray(Wo),
    )
    return out


def run_timed(x, cos, sin, Wq, Wk, Wv, Wo, iters=4):
    """Mirror of bass2jax.run_bass_via_pjrt's multi-core path, but keeps the
    compiled executable so repeated executions (device-resident inputs) can be
    wall-clocked. Returns (full_output, per_iter_seconds)."""
    import time

    import jax
    from jax.experimental.shard_map import shard_map
    from jax.sharding import Mesh, NamedSharding, PartitionSpec

    from concourse import bass2jax, mybir as mb

    T, C, NHL, BL = 2048, 2048, 4, 2
    n_cores = 8
    in_maps = prepare_in_maps(x, cos, sin, Wq, Wk, Wv, Wo, T, C, NHL, BL)
    nc = build_nc(T, C, NHL, BL)

    bass2jax.install_neuronx_cc_hook()
    partition_name = nc.partition_id_tensor.name if nc.partition_id_tensor else None
    in_names, out_names, out_avals, zero_outs = [], [], [], []
    for alloc in nc.m.functions[0].allocations:
        if not isinstance(alloc, mb.MemoryLocationSet):
            continue
        name = alloc.memorylocations[0].name
        if alloc.kind == "ExternalInput":
            if name != partition_name:
                in_names.append(name)
        elif alloc.kind == "ExternalOutput":
            shape = tuple(alloc.tensor_shape)
            dtype = mb.dt.np(alloc.dtype)
            out_names.append(name)
            out_avals.append(jax.core.ShapedArray(shape, dtype))
            zero_outs.append(np.zeros(shape, dtype))
    n_params = len(in_names)
    n_outs = len(out_avals)
    all_names = in_names + out_names
    if partition_name is not None:
        all_names = all_names + [partition_name]

    def _body(*args):
        operands = list(args)
        if partition_name is not None:
            operands.append(bass2jax.partition_id_tensor())
        outs = bass2jax._bass_exec_p.bind(
            *operands,
            out_avals=tuple(out_avals),
            in_names=tuple(all_names),
            out_names=tuple(out_names),
            lowering_input_output_aliases=(),
            sim_require_finite=True,
            sim_require_nnan=True,
            nc=nc,
        )
        return tuple(outs)

    devices = jax.devices()[:n_cores]
    mesh = Mesh(np.asarray(devices), ("core",))
    sharding = NamedSharding(mesh, PartitionSpec("core"))
    donate = tuple(range(n_params, n_params + n_outs))
    sharded = jax.jit(
        shard_map(
            _body, mesh=mesh,
            in_specs=(PartitionSpec("core"),) * (n_params + n_outs),
            out_specs=(PartitionSpec("core"),) * n_outs,
            check_rep=False,
        ),
        donate_argnums=donate, keep_unused=True,
    )

    concat_in = [
        jax.device_put(
            np.concatenate([np.asarray(m[name]) for m in in_maps], axis=0), sharding
        )
        for name in in_names
    ]
    concat_zero = lambda: [  # noqa: E731
        jax.device_put(np.zeros((n_cores * z.shape[0], *z.shape[1:]), z.dtype),
                       sharding)
        for z in zero_outs
    ]

    print("compiling + first run...")
    t0 = time.monotonic()
    out_arrs = sharded(*concat_in, *concat_zero())
    jax.block_until_ready(out_arrs)
    print(f"first call: {time.monotonic() - t0:.1f}s")
    result0 = [np.asarray(a) for a in out_arrs]

    zsets = [concat_zero() for _ in range(iters)]
    for zs in zsets:
        jax.block_until_ready(zs)
    times = []
    for it in range(iters):
        t0 = time.monotonic()
        o = sharded(*concat_in, *zsets[it])
        jax.block_until_ready(o)
        times.append(time.monotonic() - t0)
        del o

    # assemble full output from result0 (out name 'out', per-core stacked on axis 0)
    outc = result0[out_names.index("out")].reshape(n_cores, BL * T, C)
    B = x.shape[0]
    full = np.zeros((B, T, C), np.float32)
    n_hgrp = n_cores // (B // BL)
    for g in range(B // BL):
        acc = np.zeros((BL * T, C), np.float32)
        for hg in range(n_hgrp):
            acc += outc[g * n_hgrp + hg]
        full[BL * g : BL * (g + 1)] = acc.reshape(BL, T, C)
    return full, times
